# revision 16
# baseline (speedup 1.0000x reference)
"""RAFT-style CorrBlock kernel for Trainium2 (8 NeuronCores, Bass/Tile).

Full inputs: fmap1 [2,256,64,64], fmap2 [2,256,64,64], centroids_coords [2,2,64,64].
Output: [2, 324, 64, 64] f32.

Sharding: data-parallel over the B*H1*W1 query-pixel axis with 2D locality:
each batch's 4096 pixels are y-sorted into 8 blocks of 512, each block
x-sorted into 4 quartiles of 128 -> 32 cells per batch. Core c of a batch
takes 2 y-blocks (8 cells = "groups"). Every cell's pixels share a narrow
2D window of the corr volume, so the per-group matmul covers a small
[WB x HB] band per pyramid level (~1150 columns total vs ~2450 for
y-banding alone), at every level.

The band geometry (WB/HB per level) is baked as the max envelope over all
64 cells; the per-cell band *positions* are data-dependent but live only in
host-side extraction: the host writes each core's f2 input as the
concatenation of its 8 cells' zero-padded bands at program-fixed offsets.
The SPMD program therefore has fully static APs, and chunks of the
per-group matmul can span pyramid levels (3 PSUM chunks of ~FSUM/3 <= 512
per group per k-half -> 48 matmuls/core instead of 112).

Per core pipeline (group-major):
  - 2 LDWEIGHTS-phases per group: [k0: 3 chunk matmuls (start)] then
    [k1: 3 accumulating matmuls (stop)] into 3 PSUM banks,
  - PSUM -> SBUF bf16 cast copies on the Scalar engine,
  - one DRAM slab write per group (single contiguous [128, FSUM] DMA),
  - one batched indirect gather per 4-group half (16 x 128 descriptors of
    512B in ONE instruction: ~1.7us of Pool time instead of 16 x ~1.1us),
  - bilinear 4-tap combine on Vector, 4 groups per op,
  - output DMA per half; host un-permutes rows into the full output.
"""

import numpy as np
import ml_dtypes

import concourse.bass as bass
import concourse.bacc as bacc
import concourse.mybir as mybir
import concourse.tile as tile
from concourse.bass_utils import run_bass_kernel_spmd

f32 = mybir.dt.float32
bf16 = mybir.dt.bfloat16
i32 = mybir.dt.int32
BF = ml_dtypes.bfloat16
OP = mybir.AluOpType

P = 128
C = 256
NPIX = 1024        # query pixels per core
NG = 8             # groups (cells) per core
NLVL = 4
S = 9              # sample window side
FEAT = NLVL * S * S                    # 324
W_L = [64, 32, 16, 8]
BL = 256           # gathered elems per (pixel, level): 512B descriptors
NCHUNK = 3         # PSUM chunks per (group, k-half)


def _ap_view(t_ap, offset, dims):
    """Arbitrary strided view of a tile AP: dims = [[step, count], ...] free dims."""
    return bass.AP(t_ap.tensor, t_ap.offset + offset, [list(t_ap.ap[0])] + dims)


def make_plan(centroids_coords):
    """Host-side geometry: pixel->cell permutation + per-cell band positions.

    Cells: per batch, stable-argsort pixels by floor(y); 8 y-blocks of 512;
    within each, stable-argsort by floor(x); 4 x-quartiles of 128.
    Core (b*4 + c) owns y-blocks {2c, 2c+1}; its group G = (yb%2)*4 + xq.
    """
    cc = np.asarray(centroids_coords, dtype=np.float32)  # [2, 2, 64, 64]
    ccf = cc.reshape(2, 2, 4096)

    core_pix = [np.empty(NPIX, dtype=np.int64) for _ in range(8)]
    for b in range(2):
        x = ccf[b, 0]
        y = ccf[b, 1]
        ps = np.argsort(np.floor(y), kind="stable")
        for yb in range(8):
            blk = ps[yb * 512:(yb + 1) * 512]
            blk = blk[np.argsort(np.floor(x[blk]), kind="stable")]
            core = b * 4 + yb // 2
            for xq in range(4):
                g = (yb % 2) * 4 + xq
                core_pix[core][g * P:(g + 1) * P] = blk[xq * P:(xq + 1) * P]

    # per (core, group, level) band needs; envelope over all cells
    WB = np.zeros(NLVL, dtype=np.int64)
    HB = np.zeros(NLVL, dtype=np.int64)
    x0s = np.zeros((8, NG, NLVL), dtype=np.int64)
    y0s = np.zeros((8, NG, NLVL), dtype=np.int64)
    for core in range(8):
        b = core // 4
        for g in range(NG):
            pix = core_pix[core][g * P:(g + 1) * P]
            for l in range(NLVL):
                inv = 1.0 / (1 << l)
                x0 = np.floor(ccf[b, 0, pix] * inv).astype(np.int64)
                y0 = np.floor(ccf[b, 1, pix] * inv).astype(np.int64)
                x0s[core, g, l] = x0.min()
                y0s[core, g, l] = y0.min()
                WB[l] = max(WB[l], x0.max() - x0.min() + 10)
                HB[l] = max(HB[l], y0.max() - y0.min() + 10)
    assert HB[0] <= 27, "gather block must fit 256 elems"

    # band start (xs, ys) per cell, clamped so the slice stays inside the
    # padded plane (x in [-4, W+5), same for y)
    WBi = [int(w) for w in WB]
    HBi = [int(h) for h in HB]
    xs = np.zeros((8, NG, NLVL), dtype=np.int64)
    ys = np.zeros((8, NG, NLVL), dtype=np.int64)
    for l in range(NLVL):
        xs[:, :, l] = np.minimum(x0s[:, :, l] - 4, W_L[l] + 5 - WBi[l])
        ys[:, :, l] = np.minimum(y0s[:, :, l] - 4, W_L[l] + 5 - HBi[l])
        xs[:, :, l] = np.maximum(xs[:, :, l], -4)
        ys[:, :, l] = np.maximum(ys[:, :, l], -4)

    F_l = [WBi[l] * HBi[l] for l in range(NLVL)]
    FOFF = [0]
    for l in range(1, NLVL):
        FOFF.append(FOFF[-1] + F_l[l - 1])
    FSUM = FOFF[-1] + F_l[-1]

    return dict(core_pix=core_pix, WB=WBi, HB=HBi, xs=xs, ys=ys,
                F_l=F_l, FOFF=FOFF, FSUM=FSUM, ccf=ccf)


def build_bass(WB, HB):
    nc = bacc.Bacc("TRN2", target_bir_lowering=False, debug=False)

    F_l = [WB[l] * HB[l] for l in range(NLVL)]
    FSUM = sum(F_l)

    f1_d = nc.dram_tensor("f1", [C, NPIX], bf16, kind="ExternalInput")
    f2_d = [nc.dram_tensor(f"f2_{k}", [P, NG * FSUM], bf16, kind="ExternalInput")
            for k in range(2)]
    idx_d = nc.dram_tensor("idx", [P, NG * NLVL], i32, kind="ExternalInput")
    wexp_d = nc.dram_tensor("wexp", [P, NLVL * 4 * 2 * 4 * S], bf16,
                            kind="ExternalInput")
    out_d = nc.dram_tensor("out", [NPIX, FEAT], bf16, kind="ExternalOutput")
    # one slab tensor per (group, level) with levels 2+3 merged: fine-grained
    # write->gather gating, no false whole-tensor deps, fewer DMA issues
    slab_d = [[nc.dram_tensor(f"slab_{g}_0", [P * F_l[0] + BL], bf16),
               nc.dram_tensor(f"slab_{g}_1", [P * F_l[1] + BL], bf16),
               nc.dram_tensor(f"slab_{g}_23", [P * (F_l[2] + F_l[3]) + BL], bf16)]
              for g in range(NG)]

    # chunk boundaries within a group's FSUM columns (balanced, <= 512)
    bounds = [0]
    rem = FSUM
    for i in range(NCHUNK):
        w = -(-rem // (NCHUNK - i))
        assert w <= 512
        bounds.append(bounds[-1] + w)
        rem -= w
    chunks = list(zip(bounds[:-1], bounds[1:]))

    with tile.TileContext(nc) as tc:
        with (
            tc.tile_pool(name="persist", bufs=1) as pp,
            tc.tile_pool(name="grp", bufs=4) as pg,
            tc.tile_pool(name="psum", bufs=8, space="PSUM") as ps,
            tc.tile_pool(name="post", bufs=2) as po,
        ):
            # group 0's operands first on both rings so the PE starts ASAP;
            # per-(k, g) f2 tiles so a group's matmul waits only on its load
            f1t = [pp.tile([P, NPIX], bf16, tag=f"f1_{k}", name=f"f1_{k}")
                   for k in range(2)]
            f2t = {(k, g): pp.tile([P, FSUM], bf16, tag=f"f2_{k}_{g}",
                                   name=f"f2_{k}_{g}")
                   for g in range(NG) for k in range(2)}
            nc.sync.dma_start(f1t[0][:], f1_d.ap()[:P, :])
            nc.scalar.dma_start(f1t[1][:], f1_d.ap()[P:, :])
            idx = pp.tile([P, NG * NLVL], i32, tag="idx")
            wexp = pp.tile([P, NLVL * 4 * 2 * 4 * S], bf16, tag="wexp")
            for g in range(NG):
                ring = nc.sync if g % 2 == 0 else nc.scalar
                for k in range(2):
                    ring.dma_start(f2t[(k, g)][:],
                                   f2_d[k].ap()[:, g * FSUM:(g + 1) * FSUM])
                if g == 0:
                    nc.sync.dma_start(idx[:], idx_d.ap())
                elif g == 1:
                    nc.scalar.dma_start(wexp[:], wexp_d.ap())

            # one band tile per (half, level): 4 group-slices each, so a
            # combine fires as soon as its own 4 gathers land
            band = [[pp.tile([P, 4 * BL], bf16, tag=f"band_{h}_{l}",
                             name=f"band_{h}_{l}") for l in range(NLVL)]
                    for h in range(2)]
            featsH = [pp.tile([P, 4 * FEAT], bf16, tag=f"feats_{h}",
                              name=f"feats_{h}") for h in range(2)]

            def gather(g, l):
                # HW indirect DMA: ONE offset per partition, contiguous run
                nc.gpsimd.indirect_dma_start(
                    out=band[g // 4][l][:, (g % 4) * BL:(g % 4 + 1) * BL],
                    out_offset=None,
                    in_=slab_d[g][min(l, 2)].ap()[:, None],
                    in_offset=bass.IndirectOffsetOnAxis(
                        ap=idx[:, (g // 4) * 16 + (g % 4) * NLVL + l:
                               (g // 4) * 16 + (g % 4) * NLVL + l + 1], axis=0),
                    element_offset=0,
                )

            def combine(h, l):
                """Weighted 4-tap combine for the 4 groups of half h, level l."""
                hb = HB[l]
                tA = po.tile([P, 4 * S * S], bf16, tag="tA", name=f"tA_{h}_{l}")
                tB = po.tile([P, 4 * S * S], bf16, tag="tB", name=f"tB_{h}_{l}")
                tAv = tA[:].rearrange("p (g a b) -> p g a b", a=S, b=S)
                tBv = tB[:].rearrange("p (g a b) -> p g a b", a=S, b=S)

                def pv(a, b):
                    return _ap_view(
                        band[h][l][:], b * hb + a,
                        [[BL, 4], [hb, S], [1, S]],
                    )

                def wv(ab):
                    return _ap_view(
                        wexp[:], (((l * 4 + ab) * 2 + h) * 4) * S,
                        [[S, 4], [0, S], [1, S]],
                    )

                ov = _ap_view(featsH[h][:], l * S * S,
                              [[FEAT, 4], [S, S], [1, S]])
                nc.vector.tensor_tensor(out=tAv, in0=pv(0, 0), in1=wv(0), op=OP.mult)
                nc.vector.tensor_tensor(out=tBv, in0=pv(0, 1), in1=wv(1), op=OP.mult)
                nc.vector.tensor_tensor(out=tAv, in0=tAv, in1=tBv, op=OP.add)
                nc.vector.tensor_tensor(out=tBv, in0=pv(1, 0), in1=wv(2), op=OP.mult)
                nc.vector.tensor_tensor(out=tAv, in0=tAv, in1=tBv, op=OP.add)
                nc.vector.tensor_tensor(out=tBv, in0=pv(1, 1), in1=wv(3), op=OP.mult)
                nc.vector.tensor_tensor(out=ov, in0=tAv, in1=tBv, op=OP.add)

            def ship(h):
                (nc.sync if h == 0 else nc.scalar).dma_start(
                    out_d.ap().rearrange("(g p) f -> p g f", p=P)[:, 4 * h:4 * h + 4, :],
                    featsH[h][:].rearrange("p (g f) -> p g f", f=FEAT),
                )

            FOFF = [0]
            for l in range(1, NLVL):
                FOFF.append(FOFF[-1] + F_l[l - 1])
            # slab-write units: (corr col range, slab tensor idx, levels)
            WUNITS = [(FOFF[0], FOFF[0] + F_l[0], 0, (0,)),
                      (FOFF[1], FOFF[1] + F_l[1], 1, (1,)),
                      (FOFF[2], FOFF[2] + F_l[2] + F_l[3], 2, (2, 3))]

            ncopy = 0
            for g in range(NG):
                corr = pg.tile([P, FSUM], bf16, tag="corr", name=f"corr_{g}")
                wu_done = 0
                for ci, (c0, c1) in enumerate(chunks):
                    pt = ps.tile([P, 512], f32, tag="mm", name=f"mm_{g}_{ci}")
                    for k in range(2):
                        nc.tensor.matmul(
                            out=pt[:, :c1 - c0],
                            lhsT=f1t[k][:, g * P:(g + 1) * P],
                            rhs=f2t[(k, g)][:, c0:c1],
                            start=(k == 0),
                            stop=(k == 1),
                        )
                    if ncopy % 3 == 2:
                        nc.vector.tensor_copy(out=corr[:, c0:c1],
                                              in_=pt[:, :c1 - c0])
                    else:
                        nc.scalar.copy(out=corr[:, c0:c1], in_=pt[:, :c1 - c0])
                    ncopy += 1
                    # slab-write + gather each unit as soon as its columns
                    # are fully copied, to feed the Pool gather stream early
                    while wu_done < 3 and WUNITS[wu_done][1] <= c1:
                        u0, u1, si, lvls = WUNITS[wu_done]
                        weng = nc.sync if (g + wu_done) % 2 == 0 else nc.scalar
                        weng.dma_start(
                            slab_d[g][si].ap()[:P * (u1 - u0)].rearrange(
                                "(p f) -> p f", f=u1 - u0),
                            corr[:, u0:u1])
                        for l in lvls:
                            gather(g, l)
                            if g % 4 == 3:
                                combine(g // 4, l)
                        wu_done += 1
                if g == 3:
                    ship(0)
                elif g == 7:
                    ship(1)

    nc.compile()
    return nc


_NC_CACHE = {}
LAST_PLAN = None


def _get_nc(plan):
    key = (tuple(plan["WB"]), tuple(plan["HB"]))
    if key not in _NC_CACHE:
        _NC_CACHE[key] = build_bass(plan["WB"], plan["HB"])
    return _NC_CACHE[key]


def make_in_maps(fmap1, fmap2, centroids_coords, plan=None):
    global LAST_PLAN
    if plan is None:
        plan = make_plan(centroids_coords)
    LAST_PLAN = plan
    WB, HB, F_l, FOFF, FSUM = (plan["WB"], plan["HB"], plan["F_l"],
                               plan["FOFF"], plan["FSUM"])

    fmap1 = np.asarray(fmap1, dtype=np.float32)
    fmap2 = np.asarray(fmap2, dtype=np.float32)

    # padded f2 pyramid planes per batch, x-major: [C, WP, HP], x/y in [-4, W+5)
    planes = []
    for b in range(2):
        cur = fmap2[b]  # [C, 64, 64] (y, x)
        lv = []
        for l in range(NLVL):
            w = W_L[l]
            padded = np.zeros((C, w + 9, w + 9), dtype=np.float32)
            padded[:, 4:4 + w, 4:4 + w] = cur.transpose(0, 2, 1)  # [c, x, y]
            lv.append(padded)
            if l + 1 < NLVL:
                cur = cur.reshape(C, w // 2, 2, w // 2, 2).mean(axis=(2, 4))
        planes.append(lv)

    in_maps = []
    for core in range(8):
        b = core // 4
        pix = plan["core_pix"][core]                      # [1024] original ids
        f1 = (fmap1[b].reshape(C, 4096)[:, pix] * (1.0 / 16.0)).astype(BF)

        # concat band extraction: [C, NG * FSUM]
        f2full = np.empty((C, NG * FSUM), dtype=BF)
        for g in range(NG):
            for l in range(NLVL):
                xs = int(plan["xs"][core, g, l])
                ys = int(plan["ys"][core, g, l])
                bandf = planes[b][l][:, xs + 4:xs + 4 + WB[l],
                                     ys + 4:ys + 4 + HB[l]]
                f2full[:, g * FSUM + FOFF[l]:g * FSUM + FOFF[l] + F_l[l]] = (
                    bandf.reshape(C, -1).astype(BF))

        ccx = plan["ccf"][b, 0, pix]                      # [1024] f32
        ccy = plan["ccf"][b, 1, pix]
        gi = np.arange(NPIX) // P                         # group per slot
        pi = np.arange(NPIX) % P

        idx = np.zeros((P, NG * NLVL), dtype=np.int32)
        wexp = np.zeros((P, NLVL, 4, 2, 4, S), dtype=np.float32)
        for l in range(NLVL):
            inv = 1.0 / (1 << l)
            x_ = ccx * inv
            y_ = ccy * inv
            x0 = np.floor(x_).astype(np.int64)
            y0 = np.floor(y_).astype(np.int64)
            fx = (x_ - x0).astype(np.float32)
            fy = (y_ - y0).astype(np.float32)
            xs = plan["xs"][core, gi, l]
            ys = plan["ys"][core, gi, l]
            assert (x0 - xs >= 4).all() and (x0 - xs <= WB[l] - 6).all()
            assert (y0 - ys >= 4).all() and (y0 - ys <= HB[l] - 6).all()
            if l < 2:
                off = (pi * F_l[l]
                       + (x0 - xs - 4) * HB[l] + (y0 - ys - 4))
            else:
                # levels 2+3 share one slab tensor: row = [l2 band | l3 band]
                off = (pi * (F_l[2] + F_l[3]) + (F_l[2] if l == 3 else 0)
                       + (x0 - xs - 4) * HB[l] + (y0 - ys - 4))
            # column layout: (h, g%4, l)
            idx[pi, (gi // 4) * 16 + (gi % 4) * NLVL + l] = off.astype(np.int32)
            for ab, (wa, wb_) in enumerate(
                    (((1 - fy), (1 - fx)), ((1 - fy), fx),
                     (fy, (1 - fx)), (fy, fx))):
                wexp[pi, l, ab, gi // 4, gi % 4, :] = (wa * wb_)[:, None]
        in_maps.append({
            "f1": f1,
            "f2_0": np.ascontiguousarray(f2full[:P]),
            "f2_1": np.ascontiguousarray(f2full[P:]),
            "idx": idx,
            "wexp": np.ascontiguousarray(wexp.reshape(P, -1)).astype(BF),
        })
    return in_maps


def assemble(outs, plan):
    """outs: list of 8 arrays [1024, 324] -> [2, 324, 64, 64] f32."""
    full = np.empty((2, FEAT, 64, 64), dtype=np.float32)
    for b in range(2):
        feats = np.empty((4096, FEAT), dtype=np.float32)
        for c in range(4):
            feats[plan["core_pix"][b * 4 + c]] = np.asarray(
                outs[b * 4 + c], dtype=np.float32)
        full[b] = feats.reshape(64, 64, FEAT).transpose(2, 0, 1)
    return full


def kernel(fmap1, fmap2, centroids_coords, trace=False):
    plan = make_plan(centroids_coords)
    nc = _get_nc(plan)
    in_maps = make_in_maps(fmap1, fmap2, centroids_coords, plan)
    try:
        res = run_bass_kernel_spmd(nc, in_maps, core_ids=list(range(8)), trace=trace)
    except ModuleNotFoundError:
        res = run_bass_kernel_spmd(nc, in_maps, core_ids=list(range(8)), trace=False)
    out = assemble([r["out"] for r in res.results], plan)
    if trace:
        kernel.last_result = res
    return out


# revision 18
# speedup vs baseline: 1.0044x; 1.0044x over previous
"""RAFT-style CorrBlock kernel for Trainium2 (8 NeuronCores, Bass/Tile).

Full inputs: fmap1 [2,256,64,64], fmap2 [2,256,64,64], centroids_coords [2,2,64,64].
Output: [2, 324, 64, 64] f32.

Sharding: data-parallel over the B*H1*W1 query-pixel axis with 2D locality:
each batch's 4096 pixels are y-sorted into 8 blocks of 512, each block
x-sorted into 4 quartiles of 128 -> 32 cells per batch. Core c of a batch
takes 2 y-blocks (8 cells = "groups"). Every cell's pixels share a narrow
2D window of the corr volume, so the per-group matmul covers a small
[WB x HB] band per pyramid level (~1150 columns total vs ~2450 for
y-banding alone), at every level.

The band geometry (WB/HB per level) is baked as the max envelope over all
64 cells; the per-cell band *positions* are data-dependent but live only in
host-side extraction: the host writes each core's f2 input as the
concatenation of its 8 cells' zero-padded bands at program-fixed offsets.
The SPMD program therefore has fully static APs, and chunks of the
per-group matmul can span pyramid levels (3 PSUM chunks of ~FSUM/3 <= 512
per group per k-half -> 48 matmuls/core instead of 112).

Per core pipeline (group-major):
  - 2 LDWEIGHTS-phases per group: [k0: 3 chunk matmuls (start)] then
    [k1: 3 accumulating matmuls (stop)] into 3 PSUM banks,
  - PSUM -> SBUF bf16 cast copies on the Scalar engine,
  - one DRAM slab write per group (single contiguous [128, FSUM] DMA),
  - one batched indirect gather per 4-group half (16 x 128 descriptors of
    512B in ONE instruction: ~1.7us of Pool time instead of 16 x ~1.1us),
  - bilinear 4-tap combine on Vector, 4 groups per op,
  - output DMA per half; host un-permutes rows into the full output.
"""

import numpy as np
import ml_dtypes

import concourse.bass as bass
import concourse.bacc as bacc
import concourse.mybir as mybir
import concourse.tile as tile
from concourse.bass_utils import run_bass_kernel_spmd

f32 = mybir.dt.float32
bf16 = mybir.dt.bfloat16
i32 = mybir.dt.int32
BF = ml_dtypes.bfloat16
OP = mybir.AluOpType

P = 128
C = 256
NPIX = 1024        # query pixels per core
NG = 8             # groups (cells) per core
NLVL = 4
S = 9              # sample window side
FEAT = NLVL * S * S                    # 324
W_L = [64, 32, 16, 8]
BL = 256           # gathered elems per (pixel, level): 512B descriptors
NCHUNK = 3         # PSUM chunks per (group, k-half)


def _ap_view(t_ap, offset, dims):
    """Arbitrary strided view of a tile AP: dims = [[step, count], ...] free dims."""
    return bass.AP(t_ap.tensor, t_ap.offset + offset, [list(t_ap.ap[0])] + dims)


def make_plan(centroids_coords):
    """Host-side geometry: pixel->cell permutation + per-cell band positions.

    Cells: per batch, stable-argsort pixels by floor(y); 8 y-blocks of 512;
    within each, stable-argsort by floor(x); 4 x-quartiles of 128.
    Core (b*4 + c) owns y-blocks {2c, 2c+1}; its group G = (yb%2)*4 + xq.
    """
    cc = np.asarray(centroids_coords, dtype=np.float32)  # [2, 2, 64, 64]
    ccf = cc.reshape(2, 2, 4096)

    core_pix = [np.empty(NPIX, dtype=np.int64) for _ in range(8)]
    for b in range(2):
        x = ccf[b, 0]
        y = ccf[b, 1]
        ps = np.argsort(np.floor(y), kind="stable")
        for yb in range(8):
            blk = ps[yb * 512:(yb + 1) * 512]
            blk = blk[np.argsort(np.floor(x[blk]), kind="stable")]
            core = b * 4 + yb // 2
            for xq in range(4):
                g = (yb % 2) * 4 + xq
                core_pix[core][g * P:(g + 1) * P] = blk[xq * P:(xq + 1) * P]

    # per (core, group, level) band needs; envelope over all cells
    WB = np.zeros(NLVL, dtype=np.int64)
    HB = np.zeros(NLVL, dtype=np.int64)
    x0s = np.zeros((8, NG, NLVL), dtype=np.int64)
    y0s = np.zeros((8, NG, NLVL), dtype=np.int64)
    for core in range(8):
        b = core // 4
        for g in range(NG):
            pix = core_pix[core][g * P:(g + 1) * P]
            for l in range(NLVL):
                inv = 1.0 / (1 << l)
                x0 = np.floor(ccf[b, 0, pix] * inv).astype(np.int64)
                y0 = np.floor(ccf[b, 1, pix] * inv).astype(np.int64)
                x0s[core, g, l] = x0.min()
                y0s[core, g, l] = y0.min()
                WB[l] = max(WB[l], x0.max() - x0.min() + 10)
                HB[l] = max(HB[l], y0.max() - y0.min() + 10)
    assert HB[0] <= 27, "gather block must fit 256 elems"

    # band start (xs, ys) per cell, clamped so the slice stays inside the
    # padded plane (x in [-4, W+5), same for y)
    WBi = [int(w) for w in WB]
    HBi = [int(h) for h in HB]
    xs = np.zeros((8, NG, NLVL), dtype=np.int64)
    ys = np.zeros((8, NG, NLVL), dtype=np.int64)
    for l in range(NLVL):
        xs[:, :, l] = np.minimum(x0s[:, :, l] - 4, W_L[l] + 5 - WBi[l])
        ys[:, :, l] = np.minimum(y0s[:, :, l] - 4, W_L[l] + 5 - HBi[l])
        xs[:, :, l] = np.maximum(xs[:, :, l], -4)
        ys[:, :, l] = np.maximum(ys[:, :, l], -4)

    F_l = [WBi[l] * HBi[l] for l in range(NLVL)]
    FOFF = [0]
    for l in range(1, NLVL):
        FOFF.append(FOFF[-1] + F_l[l - 1])
    FSUM = FOFF[-1] + F_l[-1]

    return dict(core_pix=core_pix, WB=WBi, HB=HBi, xs=xs, ys=ys,
                F_l=F_l, FOFF=FOFF, FSUM=FSUM, ccf=ccf)


def build_bass(WB, HB):
    nc = bacc.Bacc("TRN2", target_bir_lowering=False, debug=False)

    F_l = [WB[l] * HB[l] for l in range(NLVL)]
    FSUM = sum(F_l)

    f1_d = nc.dram_tensor("f1", [C, NPIX], bf16, kind="ExternalInput")
    f2_d = [nc.dram_tensor(f"f2_{k}", [P, NG * FSUM], bf16, kind="ExternalInput")
            for k in range(2)]
    idx_d = nc.dram_tensor("idx", [P, NG * NLVL], i32, kind="ExternalInput")
    wexp_d = nc.dram_tensor("wexp", [P, NLVL * 4 * 2 * 4 * S], bf16,
                            kind="ExternalInput")
    out_d = nc.dram_tensor("out", [NPIX, FEAT], bf16, kind="ExternalOutput")
    # one slab tensor per (group, level) with levels 2+3 merged: fine-grained
    # write->gather gating, no false whole-tensor deps, fewer DMA issues
    slab_d = [[nc.dram_tensor(f"slab_{g}_0", [P * F_l[0] + BL], bf16),
               nc.dram_tensor(f"slab_{g}_1", [P * F_l[1] + BL], bf16),
               nc.dram_tensor(f"slab_{g}_23", [P * (F_l[2] + F_l[3]) + BL], bf16)]
              for g in range(NG)]

    # chunk boundaries within a group's FSUM columns (balanced, <= 512)
    bounds = [0]
    rem = FSUM
    for i in range(NCHUNK):
        w = -(-rem // (NCHUNK - i))
        assert w <= 512
        bounds.append(bounds[-1] + w)
        rem -= w
    chunks = list(zip(bounds[:-1], bounds[1:]))

    with tile.TileContext(nc) as tc:
        with (
            tc.tile_pool(name="persist", bufs=1) as pp,
            tc.tile_pool(name="grp", bufs=4) as pg,
            tc.tile_pool(name="psum", bufs=8, space="PSUM") as ps,
            tc.tile_pool(name="post", bufs=2) as po,
        ):
            # groups 0/1 load as small DMAs first (PE starts ~9us); the rest
            # as one big DMA per k-half so neither SEQ queue stalls on HWDGE
            # ring credits (which would starve the PSUM copies behind them)
            f1t = [pp.tile([P, NPIX], bf16, tag=f"f1_{k}", name=f"f1_{k}")
                   for k in range(2)]
            f2t = {(k, g): pp.tile([P, FSUM], bf16, tag=f"f2_{k}_{g}",
                                   name=f"f2_{k}_{g}")
                   for g in range(2) for k in range(2)}
            f2r = [pp.tile([P, 6 * FSUM], bf16, tag=f"f2r_{k}", name=f"f2r_{k}")
                   for k in range(2)]
            nc.sync.dma_start(f1t[0][:], f1_d.ap()[:P, :])
            nc.scalar.dma_start(f1t[1][:], f1_d.ap()[P:, :])
            idx = pp.tile([P, NG * NLVL], i32, tag="idx")
            wexp = pp.tile([P, NLVL * 4 * 2 * 4 * S], bf16, tag="wexp")
            for g in range(2):
                for k in range(2):
                    (nc.sync if k == 0 else nc.scalar).dma_start(
                        f2t[(k, g)][:],
                        f2_d[k].ap()[:, g * FSUM:(g + 1) * FSUM])
            nc.sync.dma_start(idx[:], idx_d.ap())
            nc.scalar.dma_start(wexp[:], wexp_d.ap())
            for k in range(2):
                (nc.sync if k == 0 else nc.scalar).dma_start(
                    f2r[k][:], f2_d[k].ap()[:, 2 * FSUM:])

            def rhs_ap(k, g, c0, c1):
                if g < 2:
                    return f2t[(k, g)][:, c0:c1]
                return f2r[k][:, (g - 2) * FSUM + c0:(g - 2) * FSUM + c1]

            # one band tile per (half, level): 4 group-slices each, so a
            # combine fires as soon as its own 4 gathers land
            band = [[pp.tile([P, 4 * BL], bf16, tag=f"band_{h}_{l}",
                             name=f"band_{h}_{l}") for l in range(NLVL)]
                    for h in range(2)]
            featsH = [pp.tile([P, 4 * FEAT], bf16, tag=f"feats_{h}",
                              name=f"feats_{h}") for h in range(2)]

            def gather(g, l):
                # HW indirect DMA: ONE offset per partition, contiguous run
                nc.gpsimd.indirect_dma_start(
                    out=band[g // 4][l][:, (g % 4) * BL:(g % 4 + 1) * BL],
                    out_offset=None,
                    in_=slab_d[g][min(l, 2)].ap()[:, None],
                    in_offset=bass.IndirectOffsetOnAxis(
                        ap=idx[:, (g // 4) * 16 + (g % 4) * NLVL + l:
                               (g // 4) * 16 + (g % 4) * NLVL + l + 1], axis=0),
                    element_offset=0,
                )

            def combine(h, l):
                """Weighted 4-tap combine for the 4 groups of half h, level l."""
                hb = HB[l]
                tA = po.tile([P, 4 * S * S], bf16, tag="tA", name=f"tA_{h}_{l}")
                tB = po.tile([P, 4 * S * S], bf16, tag="tB", name=f"tB_{h}_{l}")
                tAv = tA[:].rearrange("p (g a b) -> p g a b", a=S, b=S)
                tBv = tB[:].rearrange("p (g a b) -> p g a b", a=S, b=S)

                def pv(a, b):
                    return _ap_view(
                        band[h][l][:], b * hb + a,
                        [[BL, 4], [hb, S], [1, S]],
                    )

                def wv(ab):
                    return _ap_view(
                        wexp[:], (((l * 4 + ab) * 2 + h) * 4) * S,
                        [[S, 4], [0, S], [1, S]],
                    )

                ov = _ap_view(featsH[h][:], l * S * S,
                              [[FEAT, 4], [S, S], [1, S]])
                nc.vector.tensor_tensor(out=tAv, in0=pv(0, 0), in1=wv(0), op=OP.mult)
                nc.vector.tensor_tensor(out=tBv, in0=pv(0, 1), in1=wv(1), op=OP.mult)
                nc.vector.tensor_tensor(out=tAv, in0=tAv, in1=tBv, op=OP.add)
                nc.vector.tensor_tensor(out=tBv, in0=pv(1, 0), in1=wv(2), op=OP.mult)
                nc.vector.tensor_tensor(out=tAv, in0=tAv, in1=tBv, op=OP.add)
                nc.vector.tensor_tensor(out=tBv, in0=pv(1, 1), in1=wv(3), op=OP.mult)
                nc.vector.tensor_tensor(out=ov, in0=tAv, in1=tBv, op=OP.add)

            def ship(h):
                (nc.sync if h == 0 else nc.scalar).dma_start(
                    out_d.ap().rearrange("(g p) f -> p g f", p=P)[:, 4 * h:4 * h + 4, :],
                    featsH[h][:].rearrange("p (g f) -> p g f", f=FEAT),
                )

            FOFF = [0]
            for l in range(1, NLVL):
                FOFF.append(FOFF[-1] + F_l[l - 1])
            # slab-write units: (corr col range, slab tensor idx, levels)
            WUNITS = [(FOFF[0], FOFF[0] + F_l[0], 0, (0,)),
                      (FOFF[1], FOFF[1] + F_l[1], 1, (1,)),
                      (FOFF[2], FOFF[2] + F_l[2] + F_l[3], 2, (2, 3))]

            ncopy = 0
            for g in range(NG):
                corr = pg.tile([P, FSUM], bf16, tag="corr", name=f"corr_{g}")
                wu_done = 0
                for ci, (c0, c1) in enumerate(chunks):
                    pt = ps.tile([P, 512], f32, tag="mm", name=f"mm_{g}_{ci}")
                    for k in range(2):
                        nc.tensor.matmul(
                            out=pt[:, :c1 - c0],
                            lhsT=f1t[k][:, g * P:(g + 1) * P],
                            rhs=rhs_ap(k, g, c0, c1),
                            start=(k == 0),
                            stop=(k == 1),
                        )
                    if ncopy % 3 == 2:
                        nc.vector.tensor_copy(out=corr[:, c0:c1],
                                              in_=pt[:, :c1 - c0])
                    else:
                        nc.scalar.copy(out=corr[:, c0:c1], in_=pt[:, :c1 - c0])
                    ncopy += 1
                    # slab-write + gather each unit as soon as its columns
                    # are fully copied, to feed the Pool gather stream early
                    while wu_done < 3 and WUNITS[wu_done][1] <= c1:
                        u0, u1, si, lvls = WUNITS[wu_done]
                        weng = nc.sync if (g + wu_done) % 2 == 0 else nc.scalar
                        weng.dma_start(
                            slab_d[g][si].ap()[:P * (u1 - u0)].rearrange(
                                "(p f) -> p f", f=u1 - u0),
                            corr[:, u0:u1])
                        for l in lvls:
                            gather(g, l)
                            if g % 4 == 3:
                                combine(g // 4, l)
                        wu_done += 1
                if g == 3:
                    ship(0)
                elif g == 7:
                    ship(1)

    nc.compile()
    return nc


_NC_CACHE = {}
LAST_PLAN = None


def _get_nc(plan):
    key = (tuple(plan["WB"]), tuple(plan["HB"]))
    if key not in _NC_CACHE:
        _NC_CACHE[key] = build_bass(plan["WB"], plan["HB"])
    return _NC_CACHE[key]


def make_in_maps(fmap1, fmap2, centroids_coords, plan=None):
    global LAST_PLAN
    if plan is None:
        plan = make_plan(centroids_coords)
    LAST_PLAN = plan
    WB, HB, F_l, FOFF, FSUM = (plan["WB"], plan["HB"], plan["F_l"],
                               plan["FOFF"], plan["FSUM"])

    fmap1 = np.asarray(fmap1, dtype=np.float32)
    fmap2 = np.asarray(fmap2, dtype=np.float32)

    # padded f2 pyramid planes per batch, x-major: [C, WP, HP], x/y in [-4, W+5)
    planes = []
    for b in range(2):
        cur = fmap2[b]  # [C, 64, 64] (y, x)
        lv = []
        for l in range(NLVL):
            w = W_L[l]
            padded = np.zeros((C, w + 9, w + 9), dtype=np.float32)
            padded[:, 4:4 + w, 4:4 + w] = cur.transpose(0, 2, 1)  # [c, x, y]
            lv.append(padded)
            if l + 1 < NLVL:
                cur = cur.reshape(C, w // 2, 2, w // 2, 2).mean(axis=(2, 4))
        planes.append(lv)

    in_maps = []
    for core in range(8):
        b = core // 4
        pix = plan["core_pix"][core]                      # [1024] original ids
        f1 = (fmap1[b].reshape(C, 4096)[:, pix] * (1.0 / 16.0)).astype(BF)

        # concat band extraction: [C, NG * FSUM]
        f2full = np.empty((C, NG * FSUM), dtype=BF)
        for g in range(NG):
            for l in range(NLVL):
                xs = int(plan["xs"][core, g, l])
                ys = int(plan["ys"][core, g, l])
                bandf = planes[b][l][:, xs + 4:xs + 4 + WB[l],
                                     ys + 4:ys + 4 + HB[l]]
                f2full[:, g * FSUM + FOFF[l]:g * FSUM + FOFF[l] + F_l[l]] = (
                    bandf.reshape(C, -1).astype(BF))

        ccx = plan["ccf"][b, 0, pix]                      # [1024] f32
        ccy = plan["ccf"][b, 1, pix]
        gi = np.arange(NPIX) // P                         # group per slot
        pi = np.arange(NPIX) % P

        idx = np.zeros((P, NG * NLVL), dtype=np.int32)
        wexp = np.zeros((P, NLVL, 4, 2, 4, S), dtype=np.float32)
        for l in range(NLVL):
            inv = 1.0 / (1 << l)
            x_ = ccx * inv
            y_ = ccy * inv
            x0 = np.floor(x_).astype(np.int64)
            y0 = np.floor(y_).astype(np.int64)
            fx = (x_ - x0).astype(np.float32)
            fy = (y_ - y0).astype(np.float32)
            xs = plan["xs"][core, gi, l]
            ys = plan["ys"][core, gi, l]
            assert (x0 - xs >= 4).all() and (x0 - xs <= WB[l] - 6).all()
            assert (y0 - ys >= 4).all() and (y0 - ys <= HB[l] - 6).all()
            if l < 2:
                off = (pi * F_l[l]
                       + (x0 - xs - 4) * HB[l] + (y0 - ys - 4))
            else:
                # levels 2+3 share one slab tensor: row = [l2 band | l3 band]
                off = (pi * (F_l[2] + F_l[3]) + (F_l[2] if l == 3 else 0)
                       + (x0 - xs - 4) * HB[l] + (y0 - ys - 4))
            # column layout: (h, g%4, l)
            idx[pi, (gi // 4) * 16 + (gi % 4) * NLVL + l] = off.astype(np.int32)
            for ab, (wa, wb_) in enumerate(
                    (((1 - fy), (1 - fx)), ((1 - fy), fx),
                     (fy, (1 - fx)), (fy, fx))):
                wexp[pi, l, ab, gi // 4, gi % 4, :] = (wa * wb_)[:, None]
        in_maps.append({
            "f1": f1,
            "f2_0": np.ascontiguousarray(f2full[:P]),
            "f2_1": np.ascontiguousarray(f2full[P:]),
            "idx": idx,
            "wexp": np.ascontiguousarray(wexp.reshape(P, -1)).astype(BF),
        })
    return in_maps


def assemble(outs, plan):
    """outs: list of 8 arrays [1024, 324] -> [2, 324, 64, 64] f32."""
    full = np.empty((2, FEAT, 64, 64), dtype=np.float32)
    for b in range(2):
        feats = np.empty((4096, FEAT), dtype=np.float32)
        for c in range(4):
            feats[plan["core_pix"][b * 4 + c]] = np.asarray(
                outs[b * 4 + c], dtype=np.float32)
        full[b] = feats.reshape(64, 64, FEAT).transpose(2, 0, 1)
    return full


def kernel(fmap1, fmap2, centroids_coords, trace=False):
    plan = make_plan(centroids_coords)
    nc = _get_nc(plan)
    in_maps = make_in_maps(fmap1, fmap2, centroids_coords, plan)
    try:
        res = run_bass_kernel_spmd(nc, in_maps, core_ids=list(range(8)), trace=trace)
    except ModuleNotFoundError:
        res = run_bass_kernel_spmd(nc, in_maps, core_ids=list(range(8)), trace=False)
    out = assemble([r["out"] for r in res.results], plan)
    if trace:
        kernel.last_result = res
    return out


# revision 20
# speedup vs baseline: 1.0052x; 1.0008x over previous
"""RAFT-style CorrBlock kernel for Trainium2 (8 NeuronCores, Bass/Tile).

Full inputs: fmap1 [2,256,64,64], fmap2 [2,256,64,64], centroids_coords [2,2,64,64].
Output: [2, 324, 64, 64] f32.

Sharding: data-parallel over the B*H1*W1 query-pixel axis with 2D locality:
each batch's 4096 pixels are y-sorted into 8 blocks of 512, each block
x-sorted into 4 quartiles of 128 -> 32 cells per batch. Core c of a batch
takes 2 y-blocks (8 cells = "groups"). Every cell's pixels share a narrow
2D window of the corr volume, so the per-group matmul covers a small
[WB x HB] band per pyramid level (~1150 columns total vs ~2450 for
y-banding alone), at every level.

The band geometry (WB/HB per level) is baked as the max envelope over all
64 cells; the per-cell band *positions* are data-dependent but live only in
host-side extraction: the host writes each core's f2 input as the
concatenation of its 8 cells' zero-padded bands at program-fixed offsets.
The SPMD program therefore has fully static APs, and chunks of the
per-group matmul can span pyramid levels (3 PSUM chunks of ~FSUM/3 <= 512
per group per k-half -> 48 matmuls/core instead of 112).

Per core pipeline (group-major):
  - 2 LDWEIGHTS-phases per group: [k0: 3 chunk matmuls (start)] then
    [k1: 3 accumulating matmuls (stop)] into 3 PSUM banks,
  - PSUM -> SBUF bf16 cast copies on the Scalar engine,
  - one DRAM slab write per group (single contiguous [128, FSUM] DMA),
  - one batched indirect gather per 4-group half (16 x 128 descriptors of
    512B in ONE instruction: ~1.7us of Pool time instead of 16 x ~1.1us),
  - bilinear 4-tap combine on Vector, 4 groups per op,
  - output DMA per half; host un-permutes rows into the full output.
"""

import numpy as np
import ml_dtypes

import concourse.bass as bass
import concourse.bacc as bacc
import concourse.mybir as mybir
import concourse.tile as tile
from concourse.bass_utils import run_bass_kernel_spmd

f32 = mybir.dt.float32
bf16 = mybir.dt.bfloat16
i32 = mybir.dt.int32
BF = ml_dtypes.bfloat16
OP = mybir.AluOpType

P = 128
C = 256
NPIX = 1024        # query pixels per core
NG = 8             # groups (cells) per core
NLVL = 4
S = 9              # sample window side
FEAT = NLVL * S * S                    # 324
W_L = [64, 32, 16, 8]
BL = 256           # gathered elems per (pixel, level): 512B descriptors
NCHUNK = 3         # PSUM chunks per (group, k-half)


def _ap_view(t_ap, offset, dims):
    """Arbitrary strided view of a tile AP: dims = [[step, count], ...] free dims."""
    return bass.AP(t_ap.tensor, t_ap.offset + offset, [list(t_ap.ap[0])] + dims)


def make_plan(centroids_coords):
    """Host-side geometry: pixel->cell permutation + per-cell band positions.

    Cells: per batch, stable-argsort pixels by floor(y); 8 y-blocks of 512;
    within each, stable-argsort by floor(x); 4 x-quartiles of 128.
    Core (b*4 + c) owns y-blocks {2c, 2c+1}; its group G = (yb%2)*4 + xq.
    """
    cc = np.asarray(centroids_coords, dtype=np.float32)  # [2, 2, 64, 64]
    ccf = cc.reshape(2, 2, 4096)

    core_pix = [np.empty(NPIX, dtype=np.int64) for _ in range(8)]
    for b in range(2):
        x = ccf[b, 0]
        y = ccf[b, 1]
        ps = np.argsort(np.floor(y), kind="stable")
        for yb in range(8):
            blk = ps[yb * 512:(yb + 1) * 512]
            blk = blk[np.argsort(np.floor(x[blk]), kind="stable")]
            core = b * 4 + yb // 2
            for xq in range(4):
                g = (yb % 2) * 4 + xq
                core_pix[core][g * P:(g + 1) * P] = blk[xq * P:(xq + 1) * P]

    # per (core, group, level) band needs; envelope over all cells
    WB = np.zeros(NLVL, dtype=np.int64)
    HB = np.zeros(NLVL, dtype=np.int64)
    x0s = np.zeros((8, NG, NLVL), dtype=np.int64)
    y0s = np.zeros((8, NG, NLVL), dtype=np.int64)
    for core in range(8):
        b = core // 4
        for g in range(NG):
            pix = core_pix[core][g * P:(g + 1) * P]
            for l in range(NLVL):
                inv = 1.0 / (1 << l)
                x0 = np.floor(ccf[b, 0, pix] * inv).astype(np.int64)
                y0 = np.floor(ccf[b, 1, pix] * inv).astype(np.int64)
                x0s[core, g, l] = x0.min()
                y0s[core, g, l] = y0.min()
                WB[l] = max(WB[l], x0.max() - x0.min() + 10)
                HB[l] = max(HB[l], y0.max() - y0.min() + 10)
    assert HB[0] <= 27, "gather block must fit 256 elems"

    # band start (xs, ys) per cell, clamped so the slice stays inside the
    # padded plane (x in [-4, W+5), same for y)
    WBi = [int(w) for w in WB]
    HBi = [int(h) for h in HB]
    xs = np.zeros((8, NG, NLVL), dtype=np.int64)
    ys = np.zeros((8, NG, NLVL), dtype=np.int64)
    for l in range(NLVL):
        xs[:, :, l] = np.minimum(x0s[:, :, l] - 4, W_L[l] + 5 - WBi[l])
        ys[:, :, l] = np.minimum(y0s[:, :, l] - 4, W_L[l] + 5 - HBi[l])
        xs[:, :, l] = np.maximum(xs[:, :, l], -4)
        ys[:, :, l] = np.maximum(ys[:, :, l], -4)

    F_l = [WBi[l] * HBi[l] for l in range(NLVL)]
    FOFF = [0]
    for l in range(1, NLVL):
        FOFF.append(FOFF[-1] + F_l[l - 1])
    FSUM = FOFF[-1] + F_l[-1]

    return dict(core_pix=core_pix, WB=WBi, HB=HBi, xs=xs, ys=ys,
                F_l=F_l, FOFF=FOFF, FSUM=FSUM, ccf=ccf)


def build_bass(WB, HB):
    nc = bacc.Bacc("TRN2", target_bir_lowering=False, debug=False)

    F_l = [WB[l] * HB[l] for l in range(NLVL)]
    FSUM = sum(F_l)

    f1_d = nc.dram_tensor("f1", [C, NPIX], bf16, kind="ExternalInput")
    f2_d = [nc.dram_tensor(f"f2_{k}", [P, NG * FSUM], bf16, kind="ExternalInput")
            for k in range(2)]
    idx_d = nc.dram_tensor("idx", [P, NG * NLVL], i32, kind="ExternalInput")
    wexp_d = nc.dram_tensor("wexp", [P, NLVL * 4 * 2 * 4 * S], bf16,
                            kind="ExternalInput")
    out_d = nc.dram_tensor("out", [NPIX, FEAT], bf16, kind="ExternalOutput")
    # one slab tensor per (group, level) with levels 2+3 merged: fine-grained
    # write->gather gating, no false whole-tensor deps, fewer DMA issues
    slab_d = [[nc.dram_tensor(f"slab_{g}_0", [P * F_l[0] + BL], bf16),
               nc.dram_tensor(f"slab_{g}_1", [P * F_l[1] + BL], bf16),
               nc.dram_tensor(f"slab_{g}_23", [P * (F_l[2] + F_l[3]) + BL], bf16)]
              for g in range(NG)]

    # chunk boundaries within a group's FSUM columns (balanced, <= 512)
    bounds = [0]
    rem = FSUM
    for i in range(NCHUNK):
        w = -(-rem // (NCHUNK - i))
        assert w <= 512
        bounds.append(bounds[-1] + w)
        rem -= w
    chunks = list(zip(bounds[:-1], bounds[1:]))

    with tile.TileContext(nc) as tc:
        with (
            tc.tile_pool(name="persist", bufs=1) as pp,
            tc.tile_pool(name="grp", bufs=4) as pg,
            tc.tile_pool(name="psum", bufs=8, space="PSUM") as ps,
            tc.tile_pool(name="post", bufs=2) as po,
        ):
            # per-(k, g) f2 tiles, small DMAs (k0 on sync, k1 on scalar) so no
            # slab write ever queues behind a megabyte-scale load's packets
            f1t = [pp.tile([P, NPIX], bf16, tag=f"f1_{k}", name=f"f1_{k}")
                   for k in range(2)]
            f2t = {(k, g): pp.tile([P, FSUM], bf16, tag=f"f2_{k}_{g}",
                                   name=f"f2_{k}_{g}")
                   for g in range(NG) for k in range(2)}
            nc.sync.dma_start(f1t[0][:], f1_d.ap()[:P, :])
            nc.scalar.dma_start(f1t[1][:], f1_d.ap()[P:, :])
            idx = pp.tile([P, NG * NLVL], i32, tag="idx")
            wexp = pp.tile([P, NLVL * 4 * 2 * 4 * S], bf16, tag="wexp")
            for g in range(NG):
                for k in range(2):
                    (nc.sync if k == 0 else nc.scalar).dma_start(
                        f2t[(k, g)][:],
                        f2_d[k].ap()[:, g * FSUM:(g + 1) * FSUM])
                if g == 0:
                    nc.sync.dma_start(idx[:], idx_d.ap())
                elif g == 1:
                    nc.scalar.dma_start(wexp[:], wexp_d.ap())

            def rhs_ap(k, g, c0, c1):
                return f2t[(k, g)][:, c0:c1]

            # one band tile per (half, level): 4 group-slices each, so a
            # combine fires as soon as its own 4 gathers land
            band = [[pp.tile([P, 4 * BL], bf16, tag=f"band_{h}_{l}",
                             name=f"band_{h}_{l}") for l in range(NLVL)]
                    for h in range(2)]
            featsH = [pp.tile([P, 4 * FEAT], bf16, tag=f"feats_{h}",
                              name=f"feats_{h}") for h in range(2)]

            def gather(g, l):
                # HW indirect DMA: ONE offset per partition, contiguous run
                nc.gpsimd.indirect_dma_start(
                    out=band[g // 4][l][:, (g % 4) * BL:(g % 4 + 1) * BL],
                    out_offset=None,
                    in_=slab_d[g][min(l, 2)].ap()[:, None],
                    in_offset=bass.IndirectOffsetOnAxis(
                        ap=idx[:, (g // 4) * 16 + (g % 4) * NLVL + l:
                               (g // 4) * 16 + (g % 4) * NLVL + l + 1], axis=0),
                    element_offset=0,
                )

            def combine(h, l):
                """Weighted 4-tap combine for the 4 groups of half h, level l."""
                hb = HB[l]
                tA = po.tile([P, 4 * S * S], bf16, tag="tA", name=f"tA_{h}_{l}")
                tB = po.tile([P, 4 * S * S], bf16, tag="tB", name=f"tB_{h}_{l}")
                tAv = tA[:].rearrange("p (g a b) -> p g a b", a=S, b=S)
                tBv = tB[:].rearrange("p (g a b) -> p g a b", a=S, b=S)

                def pv(a, b):
                    return _ap_view(
                        band[h][l][:], b * hb + a,
                        [[BL, 4], [hb, S], [1, S]],
                    )

                def wv(ab):
                    return _ap_view(
                        wexp[:], (((l * 4 + ab) * 2 + h) * 4) * S,
                        [[S, 4], [0, S], [1, S]],
                    )

                ov = _ap_view(featsH[h][:], l * S * S,
                              [[FEAT, 4], [S, S], [1, S]])
                nc.vector.tensor_tensor(out=tAv, in0=pv(0, 0), in1=wv(0), op=OP.mult)
                nc.vector.tensor_tensor(out=tBv, in0=pv(0, 1), in1=wv(1), op=OP.mult)
                nc.vector.tensor_tensor(out=tAv, in0=tAv, in1=tBv, op=OP.add)
                nc.vector.tensor_tensor(out=tBv, in0=pv(1, 0), in1=wv(2), op=OP.mult)
                nc.vector.tensor_tensor(out=tAv, in0=tAv, in1=tBv, op=OP.add)
                nc.vector.tensor_tensor(out=tBv, in0=pv(1, 1), in1=wv(3), op=OP.mult)
                nc.vector.tensor_tensor(out=ov, in0=tAv, in1=tBv, op=OP.add)

            def ship(h):
                (nc.sync if h == 0 else nc.scalar).dma_start(
                    out_d.ap().rearrange("(g p) f -> p g f", p=P)[:, 4 * h:4 * h + 4, :],
                    featsH[h][:].rearrange("p (g f) -> p g f", f=FEAT),
                )

            FOFF = [0]
            for l in range(1, NLVL):
                FOFF.append(FOFF[-1] + F_l[l - 1])
            # slab-write units: (corr col range, slab tensor idx, levels)
            WUNITS = [(FOFF[0], FOFF[0] + F_l[0], 0, (0,)),
                      (FOFF[1], FOFF[1] + F_l[1], 1, (1,)),
                      (FOFF[2], FOFF[2] + F_l[2] + F_l[3], 2, (2, 3))]

            ncopy = 0
            for g in range(NG):
                corr = pg.tile([P, FSUM], bf16, tag="corr", name=f"corr_{g}")
                wu_done = 0
                for ci, (c0, c1) in enumerate(chunks):
                    pt = ps.tile([P, 512], f32, tag="mm", name=f"mm_{g}_{ci}")
                    for k in range(2):
                        nc.tensor.matmul(
                            out=pt[:, :c1 - c0],
                            lhsT=f1t[k][:, g * P:(g + 1) * P],
                            rhs=rhs_ap(k, g, c0, c1),
                            start=(k == 0),
                            stop=(k == 1),
                        )
                    # all copies on DVE: the Scalar SEQ must stay free to issue
                    # slab writes the moment their columns land
                    nc.vector.tensor_copy(out=corr[:, c0:c1],
                                          in_=pt[:, :c1 - c0])
                    ncopy += 1
                    # slab-write + gather each unit as soon as its columns
                    # are fully copied, to feed the Pool gather stream early
                    while wu_done < 3 and WUNITS[wu_done][1] <= c1:
                        u0, u1, si, lvls = WUNITS[wu_done]
                        weng = nc.sync if (g + wu_done) % 2 == 0 else nc.scalar
                        weng.dma_start(
                            slab_d[g][si].ap()[:P * (u1 - u0)].rearrange(
                                "(p f) -> p f", f=u1 - u0),
                            corr[:, u0:u1])
                        for l in lvls:
                            gather(g, l)
                            if g % 4 == 3:
                                combine(g // 4, l)
                        wu_done += 1
                if g == 3:
                    ship(0)
                elif g == 7:
                    ship(1)

    nc.compile()
    return nc


_NC_CACHE = {}
LAST_PLAN = None


def _get_nc(plan):
    key = (tuple(plan["WB"]), tuple(plan["HB"]))
    if key not in _NC_CACHE:
        _NC_CACHE[key] = build_bass(plan["WB"], plan["HB"])
    return _NC_CACHE[key]


def make_in_maps(fmap1, fmap2, centroids_coords, plan=None):
    global LAST_PLAN
    if plan is None:
        plan = make_plan(centroids_coords)
    LAST_PLAN = plan
    WB, HB, F_l, FOFF, FSUM = (plan["WB"], plan["HB"], plan["F_l"],
                               plan["FOFF"], plan["FSUM"])

    fmap1 = np.asarray(fmap1, dtype=np.float32)
    fmap2 = np.asarray(fmap2, dtype=np.float32)

    # padded f2 pyramid planes per batch, x-major: [C, WP, HP], x/y in [-4, W+5)
    planes = []
    for b in range(2):
        cur = fmap2[b]  # [C, 64, 64] (y, x)
        lv = []
        for l in range(NLVL):
            w = W_L[l]
            padded = np.zeros((C, w + 9, w + 9), dtype=np.float32)
            padded[:, 4:4 + w, 4:4 + w] = cur.transpose(0, 2, 1)  # [c, x, y]
            lv.append(padded)
            if l + 1 < NLVL:
                cur = cur.reshape(C, w // 2, 2, w // 2, 2).mean(axis=(2, 4))
        planes.append(lv)

    in_maps = []
    for core in range(8):
        b = core // 4
        pix = plan["core_pix"][core]                      # [1024] original ids
        f1 = (fmap1[b].reshape(C, 4096)[:, pix] * (1.0 / 16.0)).astype(BF)

        # concat band extraction: [C, NG * FSUM]
        f2full = np.empty((C, NG * FSUM), dtype=BF)
        for g in range(NG):
            for l in range(NLVL):
                xs = int(plan["xs"][core, g, l])
                ys = int(plan["ys"][core, g, l])
                bandf = planes[b][l][:, xs + 4:xs + 4 + WB[l],
                                     ys + 4:ys + 4 + HB[l]]
                f2full[:, g * FSUM + FOFF[l]:g * FSUM + FOFF[l] + F_l[l]] = (
                    bandf.reshape(C, -1).astype(BF))

        ccx = plan["ccf"][b, 0, pix]                      # [1024] f32
        ccy = plan["ccf"][b, 1, pix]
        gi = np.arange(NPIX) // P                         # group per slot
        pi = np.arange(NPIX) % P

        idx = np.zeros((P, NG * NLVL), dtype=np.int32)
        wexp = np.zeros((P, NLVL, 4, 2, 4, S), dtype=np.float32)
        for l in range(NLVL):
            inv = 1.0 / (1 << l)
            x_ = ccx * inv
            y_ = ccy * inv
            x0 = np.floor(x_).astype(np.int64)
            y0 = np.floor(y_).astype(np.int64)
            fx = (x_ - x0).astype(np.float32)
            fy = (y_ - y0).astype(np.float32)
            xs = plan["xs"][core, gi, l]
            ys = plan["ys"][core, gi, l]
            assert (x0 - xs >= 4).all() and (x0 - xs <= WB[l] - 6).all()
            assert (y0 - ys >= 4).all() and (y0 - ys <= HB[l] - 6).all()
            if l < 2:
                off = (pi * F_l[l]
                       + (x0 - xs - 4) * HB[l] + (y0 - ys - 4))
            else:
                # levels 2+3 share one slab tensor: row = [l2 band | l3 band]
                off = (pi * (F_l[2] + F_l[3]) + (F_l[2] if l == 3 else 0)
                       + (x0 - xs - 4) * HB[l] + (y0 - ys - 4))
            # column layout: (h, g%4, l)
            idx[pi, (gi // 4) * 16 + (gi % 4) * NLVL + l] = off.astype(np.int32)
            for ab, (wa, wb_) in enumerate(
                    (((1 - fy), (1 - fx)), ((1 - fy), fx),
                     (fy, (1 - fx)), (fy, fx))):
                wexp[pi, l, ab, gi // 4, gi % 4, :] = (wa * wb_)[:, None]
        in_maps.append({
            "f1": f1,
            "f2_0": np.ascontiguousarray(f2full[:P]),
            "f2_1": np.ascontiguousarray(f2full[P:]),
            "idx": idx,
            "wexp": np.ascontiguousarray(wexp.reshape(P, -1)).astype(BF),
        })
    return in_maps


def assemble(outs, plan):
    """outs: list of 8 arrays [1024, 324] -> [2, 324, 64, 64] f32."""
    full = np.empty((2, FEAT, 64, 64), dtype=np.float32)
    for b in range(2):
        feats = np.empty((4096, FEAT), dtype=np.float32)
        for c in range(4):
            feats[plan["core_pix"][b * 4 + c]] = np.asarray(
                outs[b * 4 + c], dtype=np.float32)
        full[b] = feats.reshape(64, 64, FEAT).transpose(2, 0, 1)
    return full


def kernel(fmap1, fmap2, centroids_coords, trace=False):
    plan = make_plan(centroids_coords)
    nc = _get_nc(plan)
    in_maps = make_in_maps(fmap1, fmap2, centroids_coords, plan)
    try:
        res = run_bass_kernel_spmd(nc, in_maps, core_ids=list(range(8)), trace=trace)
    except ModuleNotFoundError:
        res = run_bass_kernel_spmd(nc, in_maps, core_ids=list(range(8)), trace=False)
    out = assemble([r["out"] for r in res.results], plan)
    if trace:
        kernel.last_result = res
    return out


# revision 23
# speedup vs baseline: 1.1223x; 1.1164x over previous
"""RAFT-style CorrBlock kernel for Trainium2 (8 NeuronCores, Bass/Tile).

Full inputs: fmap1 [2,256,64,64], fmap2 [2,256,64,64], centroids_coords [2,2,64,64].
Output: [2, 324, 64, 64] f32.

Sharding: data-parallel over the B*H1*W1 query-pixel axis with 2D locality:
each batch's 4096 pixels are y-sorted into 8 blocks of 512, each block
x-sorted into 4 quartiles of 128 -> 32 cells per batch. Core c of a batch
takes 2 y-blocks (8 cells = "groups"). Every cell's pixels share a narrow
2D window of the corr volume, so the per-group matmul covers a small
[WB x HB] band per pyramid level (~1150 columns total vs ~2450 for
y-banding alone), at every level.

The band geometry (WB/HB per level) is baked as the max envelope over all
64 cells; the per-cell band *positions* are data-dependent but live only in
host-side extraction: the host writes each core's f2 input as the
concatenation of its 8 cells' zero-padded bands at program-fixed offsets.
The SPMD program therefore has fully static APs, and chunks of the
per-group matmul can span pyramid levels (3 PSUM chunks of ~FSUM/3 <= 512
per group per k-half -> 48 matmuls/core instead of 112).

Per core pipeline (group-major):
  - 2 LDWEIGHTS-phases per group: [k0: 3 chunk matmuls (start)] then
    [k1: 3 accumulating matmuls (stop)] into 3 PSUM banks,
  - PSUM -> SBUF bf16 cast copies on the Scalar engine,
  - one DRAM slab write per group (single contiguous [128, FSUM] DMA),
  - one batched indirect gather per 4-group half (16 x 128 descriptors of
    512B in ONE instruction: ~1.7us of Pool time instead of 16 x ~1.1us),
  - bilinear 4-tap combine on Vector, 4 groups per op,
  - output DMA per half; host un-permutes rows into the full output.
"""

import numpy as np
import ml_dtypes

import concourse.bass as bass
import concourse.bacc as bacc
import concourse.mybir as mybir
import concourse.tile as tile
from concourse.bass_utils import run_bass_kernel_spmd

f32 = mybir.dt.float32
bf16 = mybir.dt.bfloat16
i32 = mybir.dt.int32
BF = ml_dtypes.bfloat16
OP = mybir.AluOpType

P = 128
C = 256
NPIX = 1024        # query pixels per core
NG = 8             # groups (cells) per core
NLVL = 4
S = 9              # sample window side
FEAT = NLVL * S * S                    # 324
W_L = [64, 32, 16, 8]
BL = 256           # gathered elems per (pixel, level): 512B descriptors
NCHUNK = 3         # PSUM chunks per (group, k-half)


def _ap_view(t_ap, offset, dims):
    """Arbitrary strided view of a tile AP: dims = [[step, count], ...] free dims."""
    return bass.AP(t_ap.tensor, t_ap.offset + offset, [list(t_ap.ap[0])] + dims)


def make_plan(centroids_coords):
    """Host-side geometry: pixel->cell permutation + per-cell band positions.

    Cells: per batch, stable-argsort pixels by floor(y); 8 y-blocks of 512;
    within each, stable-argsort by floor(x); 4 x-quartiles of 128.
    Core (b*4 + c) owns y-blocks {2c, 2c+1}; its group G = (yb%2)*4 + xq.
    """
    cc = np.asarray(centroids_coords, dtype=np.float32)  # [2, 2, 64, 64]
    ccf = cc.reshape(2, 2, 4096)

    core_pix = [np.empty(NPIX, dtype=np.int64) for _ in range(8)]
    for b in range(2):
        x = ccf[b, 0]
        y = ccf[b, 1]
        ps = np.argsort(np.floor(y), kind="stable")
        for yb in range(8):
            blk = ps[yb * 512:(yb + 1) * 512]
            blk = blk[np.argsort(np.floor(x[blk]), kind="stable")]
            core = b * 4 + yb // 2
            for xq in range(4):
                g = (yb % 2) * 4 + xq
                core_pix[core][g * P:(g + 1) * P] = blk[xq * P:(xq + 1) * P]

    # per (core, group, level) band needs; envelope over all cells
    WB = np.zeros(NLVL, dtype=np.int64)
    HB = np.zeros(NLVL, dtype=np.int64)
    x0s = np.zeros((8, NG, NLVL), dtype=np.int64)
    y0s = np.zeros((8, NG, NLVL), dtype=np.int64)
    for core in range(8):
        b = core // 4
        for g in range(NG):
            pix = core_pix[core][g * P:(g + 1) * P]
            for l in range(NLVL):
                inv = 1.0 / (1 << l)
                x0 = np.floor(ccf[b, 0, pix] * inv).astype(np.int64)
                y0 = np.floor(ccf[b, 1, pix] * inv).astype(np.int64)
                x0s[core, g, l] = x0.min()
                y0s[core, g, l] = y0.min()
                WB[l] = max(WB[l], x0.max() - x0.min() + 10)
                HB[l] = max(HB[l], y0.max() - y0.min() + 10)
    assert HB[0] <= 27, "gather block must fit 256 elems"

    # band start (xs, ys) per cell, clamped so the slice stays inside the
    # padded plane (x in [-4, W+5), same for y)
    WBi = [int(w) for w in WB]
    HBi = [int(h) for h in HB]
    xs = np.zeros((8, NG, NLVL), dtype=np.int64)
    ys = np.zeros((8, NG, NLVL), dtype=np.int64)
    for l in range(NLVL):
        xs[:, :, l] = np.minimum(x0s[:, :, l] - 4, W_L[l] + 5 - WBi[l])
        ys[:, :, l] = np.minimum(y0s[:, :, l] - 4, W_L[l] + 5 - HBi[l])
        xs[:, :, l] = np.maximum(xs[:, :, l], -4)
        ys[:, :, l] = np.maximum(ys[:, :, l], -4)

    F_l = [WBi[l] * HBi[l] for l in range(NLVL)]
    FOFF = [0]
    for l in range(1, NLVL):
        FOFF.append(FOFF[-1] + F_l[l - 1])
    FSUM = FOFF[-1] + F_l[-1]

    return dict(core_pix=core_pix, WB=WBi, HB=HBi, xs=xs, ys=ys,
                F_l=F_l, FOFF=FOFF, FSUM=FSUM, ccf=ccf)


def build_bass(WB, HB):
    nc = bacc.Bacc("TRN2", target_bir_lowering=False, debug=False)

    F_l = [WB[l] * HB[l] for l in range(NLVL)]
    FSUM = sum(F_l)

    f1_d = nc.dram_tensor("f1", [C, NPIX], bf16, kind="ExternalInput")
    f2_d = [nc.dram_tensor(f"f2_{k}", [P, NG * FSUM], bf16, kind="ExternalInput")
            for k in range(2)]
    idx_d = nc.dram_tensor("idx", [P, NG * NLVL], i32, kind="ExternalInput")
    wexp_d = nc.dram_tensor("wexp", [P, NLVL * 4 * 2 * 4 * S], bf16,
                            kind="ExternalInput")
    out_d = nc.dram_tensor("out", [NPIX, FEAT], bf16, kind="ExternalOutput")
    # one slab tensor per (group, level) with levels 2+3 merged: fine-grained
    # write->gather gating, no false whole-tensor deps, fewer DMA issues
    slab_d = [[nc.dram_tensor(f"slab_{g}_0", [P * F_l[0] + BL], bf16),
               nc.dram_tensor(f"slab_{g}_1", [P * F_l[1] + BL], bf16),
               nc.dram_tensor(f"slab_{g}_23", [P * (F_l[2] + F_l[3]) + BL], bf16)]
              for g in range(NG)]

    # chunk boundaries within a group's FSUM columns (balanced, <= 512)
    bounds = [0]
    rem = FSUM
    for i in range(NCHUNK):
        w = -(-rem // (NCHUNK - i))
        assert w <= 512
        bounds.append(bounds[-1] + w)
        rem -= w
    chunks = list(zip(bounds[:-1], bounds[1:]))

    with tile.TileContext(nc) as tc:
        with (
            tc.tile_pool(name="persist", bufs=1) as pp,
            tc.tile_pool(name="grp", bufs=4) as pg,
            tc.tile_pool(name="psum", bufs=8, space="PSUM") as ps,
            tc.tile_pool(name="post", bufs=2) as po,
        ):
            # per-(k, g) f2 tiles, small DMAs (k0 on sync, k1 on scalar) so no
            # slab write ever queues behind a megabyte-scale load's packets
            f1t = [pp.tile([P, NPIX], bf16, tag=f"f1_{k}", name=f"f1_{k}")
                   for k in range(2)]
            f2t = {(k, g): pp.tile([P, FSUM], bf16, tag=f"f2_{k}_{g}",
                                   name=f"f2_{k}_{g}")
                   for g in range(NG) for k in range(2)}
            nc.sync.dma_start(f1t[0][:], f1_d.ap()[:P, :])
            nc.scalar.dma_start(f1t[1][:], f1_d.ap()[P:, :])
            idx = pp.tile([P, NG * NLVL], i32, tag="idx")
            wexp = pp.tile([P, NLVL * 4 * 2 * 4 * S], bf16, tag="wexp")
            # sync ring carries ONLY group 0's operands + all slab writes:
            # write packets must never drain behind megabytes of load traffic
            for g in range(NG):
                for k in range(2):
                    (nc.sync if g == 0 and k == 0 else nc.scalar).dma_start(
                        f2t[(k, g)][:],
                        f2_d[k].ap()[:, g * FSUM:(g + 1) * FSUM])
                if g == 0:
                    nc.sync.dma_start(idx[:], idx_d.ap())
                elif g == 1:
                    nc.scalar.dma_start(wexp[:], wexp_d.ap())

            def rhs_ap(k, g, c0, c1):
                return f2t[(k, g)][:, c0:c1]

            # one band tile per (half, level): 4 group-slices each, so a
            # combine fires as soon as its own 4 gathers land
            band = [[pp.tile([P, 4 * BL], bf16, tag=f"band_{h}_{l}",
                             name=f"band_{h}_{l}") for l in range(NLVL)]
                    for h in range(2)]
            featsH = [pp.tile([P, 4 * FEAT], bf16, tag=f"feats_{h}",
                              name=f"feats_{h}") for h in range(2)]

            def gather(g, l):
                # HW indirect DMA: ONE offset per partition, contiguous run
                nc.gpsimd.indirect_dma_start(
                    out=band[g // 4][l][:, (g % 4) * BL:(g % 4 + 1) * BL],
                    out_offset=None,
                    in_=slab_d[g][min(l, 2)].ap()[:, None],
                    in_offset=bass.IndirectOffsetOnAxis(
                        ap=idx[:, (g // 4) * 16 + (g % 4) * NLVL + l:
                               (g // 4) * 16 + (g % 4) * NLVL + l + 1], axis=0),
                    element_offset=0,
                )

            def combine(h, l):
                """Weighted 4-tap combine for the 4 groups of half h, level l."""
                hb = HB[l]
                tA = po.tile([P, 4 * S * S], bf16, tag="tA", name=f"tA_{h}_{l}")
                tB = po.tile([P, 4 * S * S], bf16, tag="tB", name=f"tB_{h}_{l}")
                tAv = tA[:].rearrange("p (g a b) -> p g a b", a=S, b=S)
                tBv = tB[:].rearrange("p (g a b) -> p g a b", a=S, b=S)

                def pv(a, b):
                    return _ap_view(
                        band[h][l][:], b * hb + a,
                        [[BL, 4], [hb, S], [1, S]],
                    )

                def wv(ab):
                    return _ap_view(
                        wexp[:], (((l * 4 + ab) * 2 + h) * 4) * S,
                        [[S, 4], [0, S], [1, S]],
                    )

                ov = _ap_view(featsH[h][:], l * S * S,
                              [[FEAT, 4], [S, S], [1, S]])
                nc.vector.tensor_tensor(out=tAv, in0=pv(0, 0), in1=wv(0), op=OP.mult)
                nc.vector.tensor_tensor(out=tBv, in0=pv(0, 1), in1=wv(1), op=OP.mult)
                nc.vector.tensor_tensor(out=tAv, in0=tAv, in1=tBv, op=OP.add)
                nc.vector.tensor_tensor(out=tBv, in0=pv(1, 0), in1=wv(2), op=OP.mult)
                nc.vector.tensor_tensor(out=tAv, in0=tAv, in1=tBv, op=OP.add)
                nc.vector.tensor_tensor(out=tBv, in0=pv(1, 1), in1=wv(3), op=OP.mult)
                nc.vector.tensor_tensor(out=ov, in0=tAv, in1=tBv, op=OP.add)

            def ship(h):
                nc.scalar.dma_start(
                    out_d.ap().rearrange("(g p) f -> p g f", p=P)[:, 4 * h:4 * h + 4, :],
                    featsH[h][:].rearrange("p (g f) -> p g f", f=FEAT),
                )

            FOFF = [0]
            for l in range(1, NLVL):
                FOFF.append(FOFF[-1] + F_l[l - 1])
            # slab-write units: (corr col range, slab tensor idx, levels)
            WUNITS = [(FOFF[0], FOFF[0] + F_l[0], 0, (0,)),
                      (FOFF[1], FOFF[1] + F_l[1], 1, (1,)),
                      (FOFF[2], FOFF[2] + F_l[2] + F_l[3], 2, (2, 3))]

            ncopy = 0
            for g in range(NG):
                corr = pg.tile([P, FSUM], bf16, tag="corr", name=f"corr_{g}")
                wu_done = 0
                for ci, (c0, c1) in enumerate(chunks):
                    pt = ps.tile([P, 512], f32, tag="mm", name=f"mm_{g}_{ci}")
                    for k in range(2):
                        nc.tensor.matmul(
                            out=pt[:, :c1 - c0],
                            lhsT=f1t[k][:, g * P:(g + 1) * P],
                            rhs=rhs_ap(k, g, c0, c1),
                            start=(k == 0),
                            stop=(k == 1),
                        )
                    # all copies on DVE: the Scalar SEQ must stay free to issue
                    # slab writes the moment their columns land
                    nc.vector.tensor_copy(out=corr[:, c0:c1],
                                          in_=pt[:, :c1 - c0])
                    ncopy += 1
                    # slab-write + gather each unit as soon as its columns
                    # are fully copied, to feed the Pool gather stream early
                    while wu_done < 3 and WUNITS[wu_done][1] <= c1:
                        u0, u1, si, lvls = WUNITS[wu_done]
                        weng = nc.sync
                        weng.dma_start(
                            slab_d[g][si].ap()[:P * (u1 - u0)].rearrange(
                                "(p f) -> p f", f=u1 - u0),
                            corr[:, u0:u1])
                        for l in lvls:
                            gather(g, l)
                            if g % 4 == 3:
                                combine(g // 4, l)
                        wu_done += 1
                if g == 3:
                    ship(0)
                elif g == 7:
                    ship(1)

    nc.compile()
    return nc


_NC_CACHE = {}
LAST_PLAN = None


def _get_nc(plan):
    key = (tuple(plan["WB"]), tuple(plan["HB"]))
    if key not in _NC_CACHE:
        _NC_CACHE[key] = build_bass(plan["WB"], plan["HB"])
    return _NC_CACHE[key]


def make_in_maps(fmap1, fmap2, centroids_coords, plan=None):
    global LAST_PLAN
    if plan is None:
        plan = make_plan(centroids_coords)
    LAST_PLAN = plan
    WB, HB, F_l, FOFF, FSUM = (plan["WB"], plan["HB"], plan["F_l"],
                               plan["FOFF"], plan["FSUM"])

    fmap1 = np.asarray(fmap1, dtype=np.float32)
    fmap2 = np.asarray(fmap2, dtype=np.float32)

    # padded f2 pyramid planes per batch, x-major: [C, WP, HP], x/y in [-4, W+5)
    planes = []
    for b in range(2):
        cur = fmap2[b]  # [C, 64, 64] (y, x)
        lv = []
        for l in range(NLVL):
            w = W_L[l]
            padded = np.zeros((C, w + 9, w + 9), dtype=np.float32)
            padded[:, 4:4 + w, 4:4 + w] = cur.transpose(0, 2, 1)  # [c, x, y]
            lv.append(padded)
            if l + 1 < NLVL:
                cur = cur.reshape(C, w // 2, 2, w // 2, 2).mean(axis=(2, 4))
        planes.append(lv)

    in_maps = []
    for core in range(8):
        b = core // 4
        pix = plan["core_pix"][core]                      # [1024] original ids
        f1 = (fmap1[b].reshape(C, 4096)[:, pix] * (1.0 / 16.0)).astype(BF)

        # concat band extraction: [C, NG * FSUM]
        f2full = np.empty((C, NG * FSUM), dtype=BF)
        for g in range(NG):
            for l in range(NLVL):
                xs = int(plan["xs"][core, g, l])
                ys = int(plan["ys"][core, g, l])
                bandf = planes[b][l][:, xs + 4:xs + 4 + WB[l],
                                     ys + 4:ys + 4 + HB[l]]
                f2full[:, g * FSUM + FOFF[l]:g * FSUM + FOFF[l] + F_l[l]] = (
                    bandf.reshape(C, -1).astype(BF))

        ccx = plan["ccf"][b, 0, pix]                      # [1024] f32
        ccy = plan["ccf"][b, 1, pix]
        gi = np.arange(NPIX) // P                         # group per slot
        pi = np.arange(NPIX) % P

        idx = np.zeros((P, NG * NLVL), dtype=np.int32)
        wexp = np.zeros((P, NLVL, 4, 2, 4, S), dtype=np.float32)
        for l in range(NLVL):
            inv = 1.0 / (1 << l)
            x_ = ccx * inv
            y_ = ccy * inv
            x0 = np.floor(x_).astype(np.int64)
            y0 = np.floor(y_).astype(np.int64)
            fx = (x_ - x0).astype(np.float32)
            fy = (y_ - y0).astype(np.float32)
            xs = plan["xs"][core, gi, l]
            ys = plan["ys"][core, gi, l]
            assert (x0 - xs >= 4).all() and (x0 - xs <= WB[l] - 6).all()
            assert (y0 - ys >= 4).all() and (y0 - ys <= HB[l] - 6).all()
            if l < 2:
                off = (pi * F_l[l]
                       + (x0 - xs - 4) * HB[l] + (y0 - ys - 4))
            else:
                # levels 2+3 share one slab tensor: row = [l2 band | l3 band]
                off = (pi * (F_l[2] + F_l[3]) + (F_l[2] if l == 3 else 0)
                       + (x0 - xs - 4) * HB[l] + (y0 - ys - 4))
            # column layout: (h, g%4, l)
            idx[pi, (gi // 4) * 16 + (gi % 4) * NLVL + l] = off.astype(np.int32)
            for ab, (wa, wb_) in enumerate(
                    (((1 - fy), (1 - fx)), ((1 - fy), fx),
                     (fy, (1 - fx)), (fy, fx))):
                wexp[pi, l, ab, gi // 4, gi % 4, :] = (wa * wb_)[:, None]
        in_maps.append({
            "f1": f1,
            "f2_0": np.ascontiguousarray(f2full[:P]),
            "f2_1": np.ascontiguousarray(f2full[P:]),
            "idx": idx,
            "wexp": np.ascontiguousarray(wexp.reshape(P, -1)).astype(BF),
        })
    return in_maps


def assemble(outs, plan):
    """outs: list of 8 arrays [1024, 324] -> [2, 324, 64, 64] f32."""
    full = np.empty((2, FEAT, 64, 64), dtype=np.float32)
    for b in range(2):
        feats = np.empty((4096, FEAT), dtype=np.float32)
        for c in range(4):
            feats[plan["core_pix"][b * 4 + c]] = np.asarray(
                outs[b * 4 + c], dtype=np.float32)
        full[b] = feats.reshape(64, 64, FEAT).transpose(2, 0, 1)
    return full


def kernel(fmap1, fmap2, centroids_coords, trace=False):
    plan = make_plan(centroids_coords)
    nc = _get_nc(plan)
    in_maps = make_in_maps(fmap1, fmap2, centroids_coords, plan)
    try:
        res = run_bass_kernel_spmd(nc, in_maps, core_ids=list(range(8)), trace=trace)
    except ModuleNotFoundError:
        res = run_bass_kernel_spmd(nc, in_maps, core_ids=list(range(8)), trace=False)
    out = assemble([r["out"] for r in res.results], plan)
    if trace:
        kernel.last_result = res
    return out
